# revision 42
# baseline (speedup 1.0000x reference)
"""Trainium2 Bass kernel for CustomDynamicEdgeConv (gnn_message_passing).

Reference computation:
    x_i = x[tgt]; x_j = x[src]
    feat = concat([x_i, x_j - x_i], -1)            # [E, 2D]
    h    = relu(feat @ W1 + b1)                    # [E, H]
    msg  = h @ W2 + b2                             # [E, Do]
    out  = segment_sum(msg, tgt) / (deg + 1e-8)

Algebraic reformulation:
    W1 = [W1a; W1b] (row split at D)
    feat @ W1 = x_i @ (W1a - W1b) + x_j @ W1b = P[tgt] + x_j @ W1b
      with P = x @ (W1a - W1b) + b1   (node-level, [N, H])
    h_e = relu(P[tgt_e] + x[src_e] @ W1b)
    out = (S @ W2) * recip + b2 * (deg * recip),  S = segment_sum(h, tgt)

Design (v7):
  * Nodes are assigned to 128-node blocks by a degree-balancing permutation
    (host side), 16 blocks per core; each core receives exactly the edges
    whose (permuted) target lies in its range — no cross-core reduction.
  * The host pre-gathers x[src] into a dense, transposed, per-tile fp8
    layout so the device does only big sequential DMA reads.
  * Q = x_j @ W1b runs as ONE fp8 DoubleRow matmul per 128-edge tile
    (contraction 256), accumulating in PSUM.
  * P stays SBUF-resident per block; P[tgt] is added into the same PSUM
    accumulation with a one-hot (M^T, bf16, host-DMA'd) matmul.
  * relu (PSUM -> SBUF fp8, x0.5 so S is scaled by 16) is split between
    DVE and ACT; h lands in [128, 2, H] pair tiles.
  * The segment sum runs on the PE as ONE fp8 DoubleRow matmul per PAIR of
    128-edge tiles (one-hot pair stationary, host-DMA'd fp8), accumulating
    S per 128-node block in PSUM.
  * fp8 scale trick: W1b and P are scaled by 32 (fp8e4 subnormal dodge);
    h is scaled by 16 = 32*0.5; 1/16 is folded into recip for the W2 stage.
  * Tail: S (bf16) goes through the hardware transpose DMA and is projected
    with W2 in dense bf16 matmuls — deferred by one block so the transpose
    latency hides under the next block's inner loop (no PE head-of-line
    stall).
  * Host un-permutes the output.
"""
import sys

sys.path.insert(0, "/opt/trn_rl_repo")

import heapq

import numpy as np
import ml_dtypes

import concourse.bacc as bacc
import concourse.mybir as mybir
from concourse.tile import TileContext
from concourse.bass_utils import run_bass_kernel_spmd

N = 16384        # nodes
D = 256          # input feature dim
H = 512          # hidden dim
DO = 256         # output dim
E = 262144       # edges
NCORES = 8
NPC = N // NCORES          # nodes per core (2048)
NB = NPC // 128            # 128-node blocks per core (16)

QMODE = "dr"               # "dr" | "f8" per-edge Q matmul mode
FSCALE = 32.0              # fp8 scale for W1b / P (power of two)
SSCALE = 16.0              # scale carried by h / S (relu scale = SSCALE/FSCALE)

f32 = mybir.dt.float32
bf16 = mybir.dt.bfloat16
f8e4 = mybir.dt.float8e4

_program_cache: dict = {}


def _build(tpb: int, reps: int = 1, ablate: frozenset = frozenset(),
           qmode: str = QMODE, rs: int = 240, sc_mode: str = "st",
           relu_pair: bool = False, hps_bufs: int = 4, b2zero: bool = True,
           ot_act: int = 0):
    """Build the SPMD Bass program. tpb = padded 128-edge tiles per node block.

    reps > 1 wraps the compute in a device-side loop (benchmarking only).
    ablate: subset of {"p1","h","relu","scatter","tail"} (timing only).
    rs: relu split point — DVE does cols [:rs], ACT does [rs:].
    sc_mode: "s" = scatter to S + transpose DMAs; "st" = transposed scatter
      (4 DR matmuls per pair, S^T direct, no transpose DMAs).
    relu_pair: one relu instruction per tile pair (vs one per tile).
    b2zero: emit the cheap tail (out = o_ps * recip) — valid when b2 == 0.
    """
    nc = bacc.Bacc("TRN2")
    G = NB * tpb                       # edge tiles per core
    T2 = 2 * tpb                       # tiles per 2-block DMA batch

    xt_own = nc.dram_tensor("xt_own", [D, NPC], bf16, kind="ExternalInput")
    w1d = nc.dram_tensor("w1d", [D, H], bf16, kind="ExternalInput")
    if qmode == "dr":
        w1bd = nc.dram_tensor("w1bd", [128, 2, H], f8e4, kind="ExternalInput")
    else:
        w1bd = nc.dram_tensor("w1bd", [D, H], f8e4, kind="ExternalInput")
    # slots 0,1: x[src] halves (partition = feature); slot 2: scatter
    # one-hot (partition = edge, free = target row)
    xgt = nc.dram_tensor("xgt", [128, G, 3, 128], f8e4, kind="ExternalInput")
    b1b = nc.dram_tensor("b1b", [128, H], f32, kind="ExternalInput")
    w2b = nc.dram_tensor("w2b", [128, 4, DO], bf16, kind="ExternalInput")
    b2b = nc.dram_tensor("b2b", [128, DO], f32, kind="ExternalInput")
    mtd = nc.dram_tensor("mtd", [128, G * 128], bf16, kind="ExternalInput")
    recs = nc.dram_tensor("recs", [128, NB], f32, kind="ExternalInput")
    gdeg = nc.dram_tensor("gdeg", [128, NB], f32, kind="ExternalInput")

    outd = nc.dram_tensor("outd", [NPC, DO], f32, kind="ExternalOutput")

    relu_t = mybir.ActivationFunctionType.Relu
    mult = mybir.AluOpType.mult
    add = mybir.AluOpType.add
    amax = mybir.AluOpType.max
    dr = mybir.MatmulPerfMode.DoubleRow
    rscale = SSCALE / FSCALE

    with TileContext(nc) as tc:
        psum_left = 8 - (2 if sc_mode == "s" else 2) - 1 - 1
        hb = min(hps_bufs, psum_left * (1 if relu_pair else 2) // 2)
        with tc.tile_pool(name="const", bufs=1) as cpool, \
             tc.tile_pool(name="xgp", bufs=2) as xgp, \
             tc.tile_pool(name="mtp", bufs=2) as mtp, \
             tc.tile_pool(name="h2p", bufs=4) as h2p, \
             tc.tile_pool(name="pxp", bufs=3) as pxp, \
             tc.tile_pool(name="ssb", bufs=2) as ssbp, \
             tc.tile_pool(name="stt", bufs=12) as sttp, \
             tc.tile_pool(name="pop", bufs=6) as pop, \
             tc.tile_pool(name="hps", bufs=hb, space="PSUM") as hpsp, \
             tc.tile_pool(name="pac", bufs=1, space="PSUM") as pacp, \
             tc.tile_pool(name="ops", bufs=1, space="PSUM") as opsp, \
             tc.tile_pool(name="sps", bufs=2, space="PSUM") as spsp:

            def load_batch(b):
                """Issue the 2-block input DMAs for blocks (b, b+1)."""
                xg = xgp.tile([128, T2, 3, 128], f8e4, tag="xg")
                nc.sync.dma_start(xg[:], xgt[:, b * tpb:(b + 2) * tpb])
                mt_sb = mtp.tile([128, T2 * 128], bf16, tag="mt")
                nc.sync.dma_start(
                    mt_sb[:], mtd[:, b * tpb * 128:(b + 2) * tpb * 128])
                return xg, mt_sb

            prefetched = None
            w1d_sb = []
            xt_sb = []
            for kk in range(2):
                t = cpool.tile([128, H], bf16, name=f"w1d_{kk}")
                nc.sync.dma_start(t[:], w1d[kk * 128:(kk + 1) * 128, :])
                w1d_sb.append(t)
                xt = cpool.tile([128, NPC], bf16, name=f"xt_{kk}")
                nc.sync.dma_start(xt[:], xt_own[kk * 128:(kk + 1) * 128, :])
                xt_sb.append(xt)
            if qmode == "dr":
                w1b_sb = cpool.tile([128, 2, H], f8e4, name="w1b")
                nc.sync.dma_start(w1b_sb[:], w1bd[:])
            else:
                w1b_sb = []
                for kk in range(2):
                    t = cpool.tile([128, H], f8e4, name=f"w1b_{kk}")
                    nc.sync.dma_start(t[:], w1bd[kk * 128:(kk + 1) * 128, :])
                    w1b_sb.append(t)
            b1b_sb = cpool.tile([128, H], f32)
            nc.sync.dma_start(b1b_sb[:], b1b[:])
            w2_sb = cpool.tile([128, 4, DO], bf16, name="w2")
            nc.sync.dma_start(w2_sb[:], w2b[:])
            recs_sb = cpool.tile([128, NB], f32)
            nc.sync.dma_start(recs_sb[:], recs[:])
            b2b_sb = gdeg_sb = None
            if not b2zero:
                b2b_sb = cpool.tile([128, DO], f32)
                nc.sync.dma_start(b2b_sb[:], b2b[:])
                gdeg_sb = cpool.tile([128, NB], f32)
                nc.sync.dma_start(gdeg_sb[:], gdeg[:])
            zmm_sb = cpool.tile([128, 128], bf16)
            nc.vector.memset(zmm_sb[:], 0)

            if reps > 1:
                _loop = tc.For_i(0, reps, 1)
                _loop.__enter__()

            # ---- Fused block loop: P -> per-edge Q + expand -> relu ->
            # scatter (DR pairs; "s": S + transpose DMA, "st": S^T direct)
            # -> deferred W2 -> out
            if True:

                def emit_tail(b, sTs):
                    o_ps = opsp.tile([128, DO], f32, tag="o")
                    for c in range(4):
                        nc.tensor.matmul(o_ps[:], sTs[c], w2_sb[:, c],
                                         start=(c == 0), stop=(c == 3))
                    o_sb = pop.tile([128, DO], f32, tag="osb")
                    if b2zero and b % 2 < ot_act:
                        nc.scalar.mul(o_sb[:], o_ps[:], recs_sb[:, b:b + 1])
                    elif b2zero:
                        nc.vector.tensor_scalar(
                            o_sb[:], o_ps[:],
                            recs_sb[:, b:b + 1], None, mult)
                    else:
                        t1 = pop.tile([128, DO], f32, tag="t1")
                        nc.any.tensor_scalar(t1[:], b2b_sb[:],
                                             gdeg_sb[:, b:b + 1], None, mult)
                        nc.vector.scalar_tensor_tensor(
                            o_sb[:], o_ps[:], recs_sb[:, b:b + 1],
                            t1[:], mult, add)
                    nc.sync.dma_start(outd[b * 128:(b + 1) * 128, :],
                                      o_sb[:])

                def emit_sc(st_ps, s_ps, it, pm, mm, hh, stop):
                    """Emit one pair's scatter matmul(s). hh is the h2 TILE
                    (sliced here, not an AP-of-AP).

                    In "st" mode the 4 slice chains share one PSUM bank and
                    start=True resets accumulate-state at BANK granularity,
                    so the bank is zeroed by a dummy matmul up front and
                    every chain matmul runs with start=False."""
                    if sc_mode == "s":
                        rh = hh[:] if pm else hh[:, 0]
                        nc.tensor.matmul(s_ps[:], mm, rh, perf_mode=pm,
                                         start=(it == 0), stop=stop)
                    else:
                        for c in range(4):
                            lh = (hh[:, :, c * 128:(c + 1) * 128] if pm
                                  else hh[:, 0, c * 128:(c + 1) * 128])
                            nc.tensor.matmul(st_ps[:, c], lh, mm,
                                             perf_mode=pm,
                                             start=False, stop=stop,
                                             skip_group_check=True)

                pending = None
                xg = mt_sb = None
                for b in range(NB):
                    if b % 2 == 0:
                        # 2-block batched input DMAs
                        if b == 0 and prefetched is not None:
                            xg, mt_sb = prefetched
                        else:
                            xg, mt_sb = load_batch(b)
                    base = (b % 2) * tpb
                    # P for this block
                    p_own_b = pxp.tile([128, H], bf16, tag="pown")
                    if "p1" not in ablate:
                        acc = pacp.tile([128, H], f32, tag="acc")
                        nc.tensor.matmul(
                            acc[:], xt_sb[0][:, b * 128:(b + 1) * 128],
                            w1d_sb[0][:], start=True, stop=False)
                        nc.tensor.matmul(
                            acc[:], xt_sb[1][:, b * 128:(b + 1) * 128],
                            w1d_sb[1][:], start=False, stop=True)
                        if b % 2 == 0:
                            nc.vector.scalar_tensor_tensor(
                                p_own_b[:], acc[:], FSCALE, b1b_sb[:],
                                mult, add)
                        else:
                            nc.scalar.activation(
                                p_own_b[:], acc[:],
                                mybir.ActivationFunctionType.Copy,
                                scale=FSCALE)

                    if sc_mode == "s":
                        s_ps = spsp.tile([128, H], f32, tag="s")
                        st_ps = None
                    else:
                        st_ps = spsp.tile([128, 4, 128], f32, tag="s")
                        s_ps = None
                        if "scatter" not in ablate:
                            # zero the bank + set has_written everywhere
                            nc.tensor.matmul(st_ps[:], zmm_sb[:],
                                             w1d_sb[0][:],
                                             start=True, stop=False,
                                             skip_group_check=True)
                    h2 = None
                    h_ps = None
                    scs = []                # deferred scatter matmuls
                    for kk in range(tpb):
                        gi = base + kk
                        par = kk % 2
                        if relu_pair:
                            if par == 0:
                                h2 = h2p.tile([128, 2, H], f8e4, tag="h2")
                                h_ps = hpsp.tile([128, 2, H], f32,
                                                 tag="hps")
                            hp = h_ps[:, par]
                        else:
                            if par == 0:
                                h2 = h2p.tile([128, 2, H], f8e4, tag="h2")
                            h_ps = hpsp.tile([128, H], f32, tag="hps")
                            hp = h_ps[:]
                        if "h" not in ablate:
                            if qmode == "dr":
                                nc.tensor.matmul(
                                    hp, xg[:, gi, 0:2], w1b_sb[:],
                                    perf_mode=dr, start=True, stop=False)
                            else:
                                nc.tensor.matmul(hp, xg[:, gi, 0],
                                                 w1b_sb[0][:],
                                                 start=True, stop=False)
                                nc.tensor.matmul(hp, xg[:, gi, 1],
                                                 w1b_sb[1][:],
                                                 start=False, stop=False)
                            nc.tensor.matmul(
                                hp,
                                mt_sb[:, gi * 128:(gi + 1) * 128],
                                p_own_b[:],
                                start=False, stop=True)
                        last = (kk == tpb - 1)
                        if relu_pair:
                            if par == 1 or last:
                                hr = h2[:] if par == 1 else h2[:, 0:1]
                                pr = h_ps[:] if par == 1 else h_ps[:, 0:1]
                                if "relu" in ablate:
                                    nc.vector.memset(hr, 0)
                                else:
                                    nc.vector.tensor_scalar(
                                        hr[:, :, 0:rs], pr[:, :, 0:rs],
                                        rscale, 0.0, mult, amax)
                                    nc.scalar.activation(
                                        hr[:, :, rs:H], pr[:, :, rs:H],
                                        relu_t, scale=rscale)
                        else:
                            hsl = h2[:, par]
                            if "relu" in ablate:
                                nc.vector.memset(hsl, 0)
                            else:
                                nc.vector.tensor_scalar(
                                    hsl[:, 0:rs], hp[:, 0:rs],
                                    rscale, 0.0, mult, amax)
                                nc.scalar.activation(
                                    hsl[:, rs:H], hp[:, rs:H], relu_t,
                                    scale=rscale)
                        if "scatter" not in ablate:
                            # collect this pair's scatter; emit the PREVIOUS
                            # pair's now (deferred by one pair) so the PE
                            # queue never waits on this pair's relu
                            grew = False
                            if par == 1:
                                scs.append((xg[:, gi - 1:gi + 1, 2], h2,
                                            dr))
                                grew = True
                            elif last:
                                scs.append((xg[:, gi, 2], h2, None))
                                grew = True
                            if grew and len(scs) >= 2:
                                i = len(scs) - 2
                                mm, hh, pm = scs[i]
                                emit_sc(st_ps, s_ps, i, pm, mm, hh, False)
                    if "scatter" not in ablate and scs:
                        i = len(scs) - 1
                        mm, hh, pm = scs[i]
                        emit_sc(st_ps, s_ps, i, pm, mm, hh, True)
                    sTs = []
                    if sc_mode == "s":
                        s_sb = ssbp.tile([128, H], bf16, tag="ssb")
                        if "scatter" in ablate:
                            nc.vector.memset(s_sb[:], 0)
                        elif b % 2 == 0:
                            nc.vector.tensor_copy(s_sb[:], s_ps[:])
                        else:
                            nc.scalar.copy(s_sb[:], s_ps[:])
                        if "tail" not in ablate:
                            for c in range(4):
                                sT = sttp.tile([128, 128], bf16, tag="st")
                                nc.sync.dma_start(
                                    sT[:], s_sb[:, c * 128:(c + 1) * 128],
                                    transpose=True)
                                sTs.append(sT[:])
                    else:
                        sT4 = sttp.tile([128, 4, 128], bf16, tag="st")
                        if "scatter" in ablate:
                            nc.vector.memset(sT4[:], 0)
                        elif b % 2 == 0:
                            nc.vector.tensor_copy(sT4[:], st_ps[:])
                        else:
                            nc.scalar.copy(sT4[:], st_ps[:])
                        sTs = [sT4[:, c] for c in range(4)]
                    if "tail" not in ablate:
                        if pending is not None:
                            emit_tail(*pending)
                        pending = (b, sTs)
                if pending is not None and "tail" not in ablate:
                    emit_tail(*pending)

            if reps > 1:
                _loop.__exit__(None, None, None)

    nc.compile()
    return nc


def _balance_nodes(deg: np.ndarray):
    """Assign nodes to 128 blocks of exactly 128 nodes, balancing total degree.

    Returns perm[N]: perm[slot] = original node id; blocks 16c..16c+15
    belong to core c.
    """
    nblocks = N // 128
    order = np.argsort(-deg, kind="stable")
    heap = [(0, 0, blk) for blk in range(nblocks)]   # (edges, nodes, blk)
    heapq.heapify(heap)
    members = [[] for _ in range(nblocks)]
    for node in order:
        w = int(deg[node])
        stash = []
        while True:
            edges, nodes, blk = heapq.heappop(heap)
            if nodes < 128:
                break
            stash.append((edges, nodes, blk))
        members[blk].append(node)
        heapq.heappush(heap, (edges + w, nodes + 1, blk))
        for it in stash:
            heapq.heappush(heap, it)
    # Repair pass: swap nodes between over-full (>cap) and under-full
    # blocks so every block has at most cap = E/nblocks edges, which keeps
    # the padded tile count at its minimum.
    cap = E // nblocks
    cnt = np.array([sum(int(deg[n]) for n in m) for m in members])
    by_deg = [dict() for _ in range(nblocks)]   # blk -> {deg: [nodes]}
    for blk in range(nblocks):
        for n in members[blk]:
            by_deg[blk].setdefault(int(deg[n]), []).append(n)
    for _ in range(10000):
        hi = int(np.argmax(cnt))
        if cnt[hi] <= cap:
            break
        lo = int(np.argmin(cnt))
        want = min(cnt[hi] - cap, cap - cnt[lo])
        done = False
        for d in range(int(want), 0, -1):
            for da, nodes_a in by_deg[hi].items():
                if not nodes_a or (da - d) not in by_deg[lo]:
                    continue
                nodes_b = by_deg[lo][da - d]
                if not nodes_b:
                    continue
                a = nodes_a.pop()
                b = nodes_b.pop()
                by_deg[lo].setdefault(da, []).append(a)
                by_deg[hi].setdefault(da - d, []).append(b)
                cnt[hi] -= d
                cnt[lo] += d
                done = True
                break
            if done:
                break
        if not done:
            break
    perm = np.empty(N, np.int64)
    pos = 0
    for blk in range(nblocks):
        nodes = [n for lst in by_deg[blk].values() for n in lst]
        assert len(nodes) == 128
        perm[pos:pos + 128] = nodes
        pos += 128
    return perm


def _prepare(x, W1, b1, W2, b2, nn_index, qmode: str = QMODE):
    src = np.asarray(nn_index[0]).astype(np.int64)
    tgt = np.asarray(nn_index[1]).astype(np.int64)
    deg = np.bincount(tgt, minlength=N).astype(np.int64)

    perm = _balance_nodes(deg)              # slot -> node
    inv = np.empty(N, np.int64)             # node -> slot
    inv[perm] = np.arange(N)

    tslot = inv[tgt]                        # permuted targets
    deg_slot = deg[perm].astype(np.float64)
    recip_full = (1.0 / (deg_slot + 1e-8)).astype(np.float32)
    gdeg_full = (deg_slot * recip_full).astype(np.float32)

    blk = tslot >> 7                        # permuted block id (0..127)
    order = np.lexsort((src, blk))
    src_s, tslot_s, blk_s = src[order], tslot[order], blk[order]
    counts = np.bincount(blk_s, minlength=N // 128)
    starts = np.concatenate(([0], np.cumsum(counts)))
    tpb = int(np.ceil(counts.max() / 128))
    pad = tpb * 128
    G = NB * tpb

    W1 = np.asarray(W1, np.float32)
    f8 = mybir.dt.np(f8e4)
    w1d_np = (W1[:D] - W1[D:]).astype(ml_dtypes.bfloat16)
    W1b = W1[D:]
    if qmode == "dr":
        # [128, 2, H]: [p, t, :] = FSCALE * W1b[t*128+p, :]
        w1bd_np = np.ascontiguousarray(
            (W1b * FSCALE).reshape(2, 128, H).transpose(1, 0, 2)).astype(f8)
    else:
        w1bd_np = (W1b * FSCALE).astype(f8)
    b1b_np = np.tile(np.asarray(b1, np.float32)[None, :] * FSCALE, (128, 1))
    b2b_np = np.tile(np.asarray(b2, np.float32)[None, :], (128, 1))
    # [128, 4, DO]: [p, c, :] = W2[c*128+p, :]
    w2b_np = np.ascontiguousarray(
        np.asarray(W2, np.float32).reshape(4, 128, DO).transpose(1, 0, 2)
    ).astype(ml_dtypes.bfloat16)

    x_np = np.asarray(x, np.float32)
    xt_np = np.ascontiguousarray(x_np.T.astype(ml_dtypes.bfloat16))
    # transposed x for the pre-gather (quantized once, [D, N])
    xT_q = np.ascontiguousarray(x_np.T.astype(f8))

    in_maps = []
    for c in range(NCORES):
        sflat = np.zeros((NB, pad), np.int64)
        tl = np.full((NB, pad), -1.0, np.float32)
        for b in range(NB):
            g = c * NB + b               # global (permuted) block
            s, e = starts[g], starts[g + 1]
            n = e - s
            sflat[b, :n] = src_s[s:e]
            tl[b, :n] = (tslot_s[s:e] & 127).astype(np.float32)
        # pre-gathered transposed x[src] + scatter one-hot: [128, G, 3, 128]
        # slots 0,1: [p, g, t, j] = x[src[g*128+j], t*128+p]
        # slot 2:    [e, g, 2, r] = 1 if tloc of edge g*128+e == r
        A = xT_q[:, sflat.reshape(-1)]                  # [256, G*128]
        xgt_np = np.zeros((128, G, 3, 128), f8)
        xgt_np[:, :, 0:2, :] = A.reshape(2, 128, G, 128).transpose(1, 2, 0, 3)
        tlf = tl.reshape(-1)
        valid = tlf >= 0
        cols = np.arange(G * 128)
        ev, gv = (cols[valid] & 127), (cols[valid] >> 7)
        ones = np.ones(len(ev), f8)
        xgt_np[ev, gv, 2, tlf[valid].astype(np.int64)] = ones
        # M^T tiles: [128, G*128] bf16; column g*128+e one-hot at row tloc
        mtd_np = np.zeros((128, G * 128), ml_dtypes.bfloat16)
        mtd_np[tlf[valid].astype(np.int64), cols[valid]] = 1
        recs_np = np.ascontiguousarray(
            (recip_full[c * NPC:(c + 1) * NPC] / SSCALE).reshape(NB, 128).T)
        gdeg_np = np.ascontiguousarray(
            gdeg_full[c * NPC:(c + 1) * NPC].reshape(NB, 128).T)
        in_maps.append({
            "xt_own": np.ascontiguousarray(
                xt_np[:, perm[c * NPC:(c + 1) * NPC]]),
            "w1d": w1d_np, "w1bd": w1bd_np, "b1b": b1b_np,
            "w2b": w2b_np, "b2b": b2b_np,
            "xgt": xgt_np, "mtd": mtd_np,
            "recs": recs_np, "gdeg": gdeg_np,
        })
    return tpb, in_maps, perm


def kernel(x, W1, b1, W2, b2, nn_index, k=None, _trace=False, _tmpdir=None):
    tpb, in_maps, perm = _prepare(x, W1, b1, W2, b2, nn_index)
    b2zero = bool(np.all(np.asarray(b2) == 0))
    key = (tpb, QMODE, b2zero)
    if key not in _program_cache:
        _program_cache[key] = _build(tpb, qmode=QMODE, b2zero=b2zero)
    nc = _program_cache[key]
    res = run_bass_kernel_spmd(nc, in_maps, core_ids=list(range(NCORES)),
                               trace=_trace, tmpdir=_tmpdir)
    out_perm = np.concatenate([res.results[c]["outd"] for c in range(NCORES)],
                              axis=0)
    out = np.empty_like(out_perm)
    out[perm] = out_perm                    # slot s holds node perm[s]
    if _trace:
        return out.astype(np.float32), res
    return out.astype(np.float32)


# revision 44
# speedup vs baseline: 2.3184x; 2.3184x over previous
"""Trainium2 Bass kernel for CustomDynamicEdgeConv (gnn_message_passing).

Reference computation:
    x_i = x[tgt]; x_j = x[src]
    feat = concat([x_i, x_j - x_i], -1)            # [E, 2D]
    h    = relu(feat @ W1 + b1)                    # [E, H]
    msg  = h @ W2 + b2                             # [E, Do]
    out  = segment_sum(msg, tgt) / (deg + 1e-8)

Algebraic reformulation:
    W1 = [W1a; W1b] (row split at D)
    feat @ W1 = x_i @ (W1a - W1b) + x_j @ W1b = P[tgt] + x_j @ W1b
      with P = x @ (W1a - W1b) + b1   (node-level, [N, H])
    h_e = relu(P[tgt_e] + x[src_e] @ W1b)
    out = (S @ W2) * recip + b2 * (deg * recip),  S = segment_sum(h, tgt)

Design (v7):
  * Nodes are assigned to 128-node blocks by a degree-balancing permutation
    (host side), 16 blocks per core; each core receives exactly the edges
    whose (permuted) target lies in its range — no cross-core reduction.
  * The host pre-gathers x[src] into a dense, transposed, per-tile fp8
    layout so the device does only big sequential DMA reads.
  * Q = x_j @ W1b runs as ONE fp8 DoubleRow matmul per 128-edge tile
    (contraction 256), accumulating in PSUM.
  * P stays SBUF-resident per block; P[tgt] is added into the same PSUM
    accumulation with a one-hot (M^T, bf16, host-DMA'd) matmul.
  * relu (PSUM -> SBUF fp8, x0.5 so S is scaled by 16) is split between
    DVE and ACT; h lands in [128, 2, H] pair tiles.
  * The segment sum runs on the PE as ONE fp8 DoubleRow matmul per PAIR of
    128-edge tiles (one-hot pair stationary, host-DMA'd fp8), accumulating
    S per 128-node block in PSUM.
  * fp8 scale trick: W1b and P are scaled by 32 (fp8e4 subnormal dodge);
    h is scaled by 16 = 32*0.5; 1/16 is folded into recip for the W2 stage.
  * Tail: S (bf16) goes through the hardware transpose DMA and is projected
    with W2 in dense bf16 matmuls — deferred by one block so the transpose
    latency hides under the next block's inner loop (no PE head-of-line
    stall).
  * Host un-permutes the output.
"""
import sys

sys.path.insert(0, "/opt/trn_rl_repo")

import heapq

import numpy as np
import ml_dtypes

import concourse.bacc as bacc
import concourse.mybir as mybir
from concourse.tile import TileContext
from concourse.bass_utils import run_bass_kernel_spmd

N = 16384        # nodes
D = 256          # input feature dim
H = 512          # hidden dim
DO = 256         # output dim
E = 262144       # edges
NCORES = 8
NPC = N // NCORES          # nodes per core (2048)
NB = NPC // 128            # 128-node blocks per core (16)

QMODE = "dr"               # "dr" | "f8" per-edge Q matmul mode
FSCALE = 32.0              # fp8 scale for W1b / P (power of two)
SSCALE = 16.0              # scale carried by h / S (relu scale = SSCALE/FSCALE)

f32 = mybir.dt.float32
bf16 = mybir.dt.bfloat16
f8e4 = mybir.dt.float8e4

_program_cache: dict = {}


def _build(tpb: int, reps: int = 1, ablate: frozenset = frozenset(),
           qmode: str = QMODE, rs: int = 240, sc_mode: str = "st",
           relu_pair: bool = False, hps_bufs: int = 4, b2zero: bool = True,
           ot_act: int = 1, relu_alt: bool = True):
    """Build the SPMD Bass program. tpb = padded 128-edge tiles per node block.

    reps > 1 wraps the compute in a device-side loop (benchmarking only).
    ablate: subset of {"p1","h","relu","scatter","tail"} (timing only).
    rs: relu split point — DVE does cols [:rs], ACT does [rs:].
    sc_mode: "s" = scatter to S + transpose DMAs; "st" = transposed scatter
      (4 DR matmuls per pair, S^T direct, no transpose DMAs).
    relu_pair: one relu instruction per tile pair (vs one per tile).
    b2zero: emit the cheap tail (out = o_ps * recip) — valid when b2 == 0.
    """
    nc = bacc.Bacc("TRN2")
    G = NB * tpb                       # edge tiles per core
    T2 = 2 * tpb                       # tiles per 2-block DMA batch

    xt_own = nc.dram_tensor("xt_own", [D, NPC], bf16, kind="ExternalInput")
    w1d = nc.dram_tensor("w1d", [D, H], bf16, kind="ExternalInput")
    if qmode == "dr":
        w1bd = nc.dram_tensor("w1bd", [128, 2, H], f8e4, kind="ExternalInput")
    else:
        w1bd = nc.dram_tensor("w1bd", [D, H], f8e4, kind="ExternalInput")
    # slots 0,1: x[src] halves (partition = feature); slot 2: scatter
    # one-hot (partition = edge, free = target row)
    xgt = nc.dram_tensor("xgt", [128, G, 3, 128], f8e4, kind="ExternalInput")
    b1b = nc.dram_tensor("b1b", [128, H], f32, kind="ExternalInput")
    w2b = nc.dram_tensor("w2b", [128, 4, DO], bf16, kind="ExternalInput")
    b2b = nc.dram_tensor("b2b", [128, DO], f32, kind="ExternalInput")
    mtd = nc.dram_tensor("mtd", [128, G * 128], bf16, kind="ExternalInput")
    recs = nc.dram_tensor("recs", [128, NB], f32, kind="ExternalInput")
    gdeg = nc.dram_tensor("gdeg", [128, NB], f32, kind="ExternalInput")

    outd = nc.dram_tensor("outd", [NPC, DO], f32, kind="ExternalOutput")

    relu_t = mybir.ActivationFunctionType.Relu
    mult = mybir.AluOpType.mult
    add = mybir.AluOpType.add
    amax = mybir.AluOpType.max
    dr = mybir.MatmulPerfMode.DoubleRow
    rscale = SSCALE / FSCALE

    with TileContext(nc) as tc:
        psum_left = 8 - (2 if sc_mode == "s" else 2) - 1 - 1
        hb = min(hps_bufs, psum_left * (1 if relu_pair else 2) // 2)
        with tc.tile_pool(name="const", bufs=1) as cpool, \
             tc.tile_pool(name="xgp", bufs=2) as xgp, \
             tc.tile_pool(name="mtp", bufs=2) as mtp, \
             tc.tile_pool(name="h2p", bufs=4) as h2p, \
             tc.tile_pool(name="pxp", bufs=3) as pxp, \
             tc.tile_pool(name="ssb", bufs=2) as ssbp, \
             tc.tile_pool(name="stt", bufs=12) as sttp, \
             tc.tile_pool(name="pop", bufs=6) as pop, \
             tc.tile_pool(name="hps", bufs=hb, space="PSUM") as hpsp, \
             tc.tile_pool(name="pac", bufs=1, space="PSUM") as pacp, \
             tc.tile_pool(name="ops", bufs=1, space="PSUM") as opsp, \
             tc.tile_pool(name="sps", bufs=2, space="PSUM") as spsp:

            def load_batch(b):
                """Issue the 2-block input DMAs for blocks (b, b+1)."""
                xg = xgp.tile([128, T2, 3, 128], f8e4, tag="xg")
                nc.sync.dma_start(xg[:], xgt[:, b * tpb:(b + 2) * tpb])
                mt_sb = mtp.tile([128, T2 * 128], bf16, tag="mt")
                nc.sync.dma_start(
                    mt_sb[:], mtd[:, b * tpb * 128:(b + 2) * tpb * 128])
                return xg, mt_sb

            prefetched = None
            w1d_sb = []
            xt_sb = []
            for kk in range(2):
                t = cpool.tile([128, H], bf16, name=f"w1d_{kk}")
                nc.sync.dma_start(t[:], w1d[kk * 128:(kk + 1) * 128, :])
                w1d_sb.append(t)
                xt = cpool.tile([128, NPC], bf16, name=f"xt_{kk}")
                nc.sync.dma_start(xt[:], xt_own[kk * 128:(kk + 1) * 128, :])
                xt_sb.append(xt)
            if qmode == "dr":
                w1b_sb = cpool.tile([128, 2, H], f8e4, name="w1b")
                nc.sync.dma_start(w1b_sb[:], w1bd[:])
            else:
                w1b_sb = []
                for kk in range(2):
                    t = cpool.tile([128, H], f8e4, name=f"w1b_{kk}")
                    nc.sync.dma_start(t[:], w1bd[kk * 128:(kk + 1) * 128, :])
                    w1b_sb.append(t)
            b1b_sb = cpool.tile([128, H], f32)
            nc.sync.dma_start(b1b_sb[:], b1b[:])
            w2_sb = cpool.tile([128, 4, DO], bf16, name="w2")
            nc.sync.dma_start(w2_sb[:], w2b[:])
            recs_sb = cpool.tile([128, NB], f32)
            nc.sync.dma_start(recs_sb[:], recs[:])
            b2b_sb = gdeg_sb = None
            if not b2zero:
                b2b_sb = cpool.tile([128, DO], f32)
                nc.sync.dma_start(b2b_sb[:], b2b[:])
                gdeg_sb = cpool.tile([128, NB], f32)
                nc.sync.dma_start(gdeg_sb[:], gdeg[:])
            zmm_sb = cpool.tile([128, 128], bf16)
            nc.vector.memset(zmm_sb[:], 0)

            if reps > 1:
                _loop = tc.For_i(0, reps, 1)
                _loop.__enter__()

            # ---- Fused block loop: P -> per-edge Q + expand -> relu ->
            # scatter (DR pairs; "s": S + transpose DMA, "st": S^T direct)
            # -> deferred W2 -> out
            if True:

                def emit_tail(b, sTs):
                    o_ps = opsp.tile([128, DO], f32, tag="o")
                    for c in range(4):
                        nc.tensor.matmul(o_ps[:], sTs[c], w2_sb[:, c],
                                         start=(c == 0), stop=(c == 3))
                    o_sb = pop.tile([128, DO], f32, tag="osb")
                    if b2zero and b % 2 < ot_act:
                        nc.scalar.mul(o_sb[:], o_ps[:], recs_sb[:, b:b + 1])
                    elif b2zero:
                        nc.vector.tensor_scalar(
                            o_sb[:], o_ps[:],
                            recs_sb[:, b:b + 1], None, mult)
                    else:
                        t1 = pop.tile([128, DO], f32, tag="t1")
                        nc.any.tensor_scalar(t1[:], b2b_sb[:],
                                             gdeg_sb[:, b:b + 1], None, mult)
                        nc.vector.scalar_tensor_tensor(
                            o_sb[:], o_ps[:], recs_sb[:, b:b + 1],
                            t1[:], mult, add)
                    nc.sync.dma_start(outd[b * 128:(b + 1) * 128, :],
                                      o_sb[:])

                def emit_sc(st_ps, s_ps, it, pm, mm, hh, stop):
                    """Emit one pair's scatter matmul(s). hh is the h2 TILE
                    (sliced here, not an AP-of-AP).

                    In "st" mode the 4 slice chains share one PSUM bank and
                    start=True resets accumulate-state at BANK granularity,
                    so the bank is zeroed by a dummy matmul up front and
                    every chain matmul runs with start=False."""
                    if sc_mode == "s":
                        rh = hh[:] if pm else hh[:, 0]
                        nc.tensor.matmul(s_ps[:], mm, rh, perf_mode=pm,
                                         start=(it == 0), stop=stop)
                    else:
                        for c in range(4):
                            lh = (hh[:, :, c * 128:(c + 1) * 128] if pm
                                  else hh[:, 0, c * 128:(c + 1) * 128])
                            nc.tensor.matmul(st_ps[:, c], lh, mm,
                                             perf_mode=pm,
                                             start=False, stop=stop,
                                             skip_group_check=True)

                pending = None
                xg = mt_sb = None
                for b in range(NB):
                    if b % 2 == 0:
                        # 2-block batched input DMAs
                        if b == 0 and prefetched is not None:
                            xg, mt_sb = prefetched
                        else:
                            xg, mt_sb = load_batch(b)
                    base = (b % 2) * tpb
                    # P for this block
                    p_own_b = pxp.tile([128, H], bf16, tag="pown")
                    if "p1" not in ablate:
                        acc = pacp.tile([128, H], f32, tag="acc")
                        nc.tensor.matmul(
                            acc[:], xt_sb[0][:, b * 128:(b + 1) * 128],
                            w1d_sb[0][:], start=True, stop=False)
                        nc.tensor.matmul(
                            acc[:], xt_sb[1][:, b * 128:(b + 1) * 128],
                            w1d_sb[1][:], start=False, stop=True)
                        if b % 2 == 0:
                            nc.vector.scalar_tensor_tensor(
                                p_own_b[:], acc[:], FSCALE, b1b_sb[:],
                                mult, add)
                        else:
                            nc.scalar.activation(
                                p_own_b[:], acc[:],
                                mybir.ActivationFunctionType.Copy,
                                scale=FSCALE)

                    if sc_mode == "s":
                        s_ps = spsp.tile([128, H], f32, tag="s")
                        st_ps = None
                    else:
                        st_ps = spsp.tile([128, 4, 128], f32, tag="s")
                        s_ps = None
                        if "scatter" not in ablate:
                            # zero the bank + set has_written everywhere
                            nc.tensor.matmul(st_ps[:], zmm_sb[:],
                                             w1d_sb[0][:],
                                             start=True, stop=False,
                                             skip_group_check=True)
                    h2 = None
                    h_ps = None
                    scs = []                # deferred scatter matmuls
                    for kk in range(tpb):
                        gi = base + kk
                        par = kk % 2
                        if relu_pair:
                            if par == 0:
                                h2 = h2p.tile([128, 2, H], f8e4, tag="h2")
                                h_ps = hpsp.tile([128, 2, H], f32,
                                                 tag="hps")
                            hp = h_ps[:, par]
                        else:
                            if par == 0:
                                h2 = h2p.tile([128, 2, H], f8e4, tag="h2")
                            h_ps = hpsp.tile([128, H], f32, tag="hps")
                            hp = h_ps[:]
                        if "h" not in ablate:
                            if qmode == "dr":
                                nc.tensor.matmul(
                                    hp, xg[:, gi, 0:2], w1b_sb[:],
                                    perf_mode=dr, start=True, stop=False)
                            else:
                                nc.tensor.matmul(hp, xg[:, gi, 0],
                                                 w1b_sb[0][:],
                                                 start=True, stop=False)
                                nc.tensor.matmul(hp, xg[:, gi, 1],
                                                 w1b_sb[1][:],
                                                 start=False, stop=False)
                            nc.tensor.matmul(
                                hp,
                                mt_sb[:, gi * 128:(gi + 1) * 128],
                                p_own_b[:],
                                start=False, stop=True)
                        last = (kk == tpb - 1)
                        if relu_pair:
                            if par == 1 or last:
                                hr = h2[:] if par == 1 else h2[:, 0:1]
                                pr = h_ps[:] if par == 1 else h_ps[:, 0:1]
                                if "relu" in ablate:
                                    nc.vector.memset(hr, 0)
                                else:
                                    nc.vector.tensor_scalar(
                                        hr[:, :, 0:rs], pr[:, :, 0:rs],
                                        rscale, 0.0, mult, amax)
                                    nc.scalar.activation(
                                        hr[:, :, rs:H], pr[:, :, rs:H],
                                        relu_t, scale=rscale)
                        else:
                            hsl = h2[:, par]
                            if "relu" in ablate:
                                nc.vector.memset(hsl, 0)
                            elif relu_alt:
                                # whole-tile alternation: fewer, wider ops
                                if gi % 2 == 0:
                                    nc.vector.tensor_scalar(
                                        hsl, hp, rscale, 0.0, mult, amax)
                                else:
                                    nc.scalar.activation(
                                        hsl, hp, relu_t, scale=rscale)
                            else:
                                nc.vector.tensor_scalar(
                                    hsl[:, 0:rs], hp[:, 0:rs],
                                    rscale, 0.0, mult, amax)
                                nc.scalar.activation(
                                    hsl[:, rs:H], hp[:, rs:H], relu_t,
                                    scale=rscale)
                        if "scatter" not in ablate:
                            # collect this pair's scatter; emit the PREVIOUS
                            # pair's now (deferred by one pair) so the PE
                            # queue never waits on this pair's relu
                            grew = False
                            if par == 1:
                                scs.append((xg[:, gi - 1:gi + 1, 2], h2,
                                            dr))
                                grew = True
                            elif last:
                                scs.append((xg[:, gi, 2], h2, None))
                                grew = True
                            if grew and len(scs) >= 2:
                                i = len(scs) - 2
                                mm, hh, pm = scs[i]
                                emit_sc(st_ps, s_ps, i, pm, mm, hh, False)
                    if "scatter" not in ablate and scs:
                        i = len(scs) - 1
                        mm, hh, pm = scs[i]
                        emit_sc(st_ps, s_ps, i, pm, mm, hh, True)
                    sTs = []
                    if sc_mode == "s":
                        s_sb = ssbp.tile([128, H], bf16, tag="ssb")
                        if "scatter" in ablate:
                            nc.vector.memset(s_sb[:], 0)
                        elif b % 2 == 0:
                            nc.vector.tensor_copy(s_sb[:], s_ps[:])
                        else:
                            nc.scalar.copy(s_sb[:], s_ps[:])
                        if "tail" not in ablate:
                            for c in range(4):
                                sT = sttp.tile([128, 128], bf16, tag="st")
                                nc.sync.dma_start(
                                    sT[:], s_sb[:, c * 128:(c + 1) * 128],
                                    transpose=True)
                                sTs.append(sT[:])
                    else:
                        sT4 = sttp.tile([128, 4, 128], bf16, tag="st")
                        if "scatter" in ablate:
                            nc.vector.memset(sT4[:], 0)
                        elif b % 2 == 0:
                            nc.vector.tensor_copy(sT4[:], st_ps[:])
                        else:
                            nc.scalar.copy(sT4[:], st_ps[:])
                        sTs = [sT4[:, c] for c in range(4)]
                    if "tail" not in ablate:
                        if pending is not None:
                            emit_tail(*pending)
                        pending = (b, sTs)
                if pending is not None and "tail" not in ablate:
                    emit_tail(*pending)

            if reps > 1:
                _loop.__exit__(None, None, None)

    nc.compile()
    return nc


def _balance_nodes(deg: np.ndarray):
    """Assign nodes to 128 blocks of exactly 128 nodes, balancing total degree.

    Returns perm[N]: perm[slot] = original node id; blocks 16c..16c+15
    belong to core c.
    """
    nblocks = N // 128
    order = np.argsort(-deg, kind="stable")
    heap = [(0, 0, blk) for blk in range(nblocks)]   # (edges, nodes, blk)
    heapq.heapify(heap)
    members = [[] for _ in range(nblocks)]
    for node in order:
        w = int(deg[node])
        stash = []
        while True:
            edges, nodes, blk = heapq.heappop(heap)
            if nodes < 128:
                break
            stash.append((edges, nodes, blk))
        members[blk].append(node)
        heapq.heappush(heap, (edges + w, nodes + 1, blk))
        for it in stash:
            heapq.heappush(heap, it)
    # Repair pass: swap nodes between over-full (>cap) and under-full
    # blocks so every block has at most cap = E/nblocks edges, which keeps
    # the padded tile count at its minimum.
    cap = E // nblocks
    cnt = np.array([sum(int(deg[n]) for n in m) for m in members])
    by_deg = [dict() for _ in range(nblocks)]   # blk -> {deg: [nodes]}
    for blk in range(nblocks):
        for n in members[blk]:
            by_deg[blk].setdefault(int(deg[n]), []).append(n)
    for _ in range(10000):
        hi = int(np.argmax(cnt))
        if cnt[hi] <= cap:
            break
        lo = int(np.argmin(cnt))
        want = min(cnt[hi] - cap, cap - cnt[lo])
        done = False
        for d in range(int(want), 0, -1):
            for da, nodes_a in by_deg[hi].items():
                if not nodes_a or (da - d) not in by_deg[lo]:
                    continue
                nodes_b = by_deg[lo][da - d]
                if not nodes_b:
                    continue
                a = nodes_a.pop()
                b = nodes_b.pop()
                by_deg[lo].setdefault(da, []).append(a)
                by_deg[hi].setdefault(da - d, []).append(b)
                cnt[hi] -= d
                cnt[lo] += d
                done = True
                break
            if done:
                break
        if not done:
            break
    perm = np.empty(N, np.int64)
    pos = 0
    for blk in range(nblocks):
        nodes = [n for lst in by_deg[blk].values() for n in lst]
        assert len(nodes) == 128
        perm[pos:pos + 128] = nodes
        pos += 128
    return perm


def _prepare(x, W1, b1, W2, b2, nn_index, qmode: str = QMODE):
    src = np.asarray(nn_index[0]).astype(np.int64)
    tgt = np.asarray(nn_index[1]).astype(np.int64)
    deg = np.bincount(tgt, minlength=N).astype(np.int64)

    perm = _balance_nodes(deg)              # slot -> node
    inv = np.empty(N, np.int64)             # node -> slot
    inv[perm] = np.arange(N)

    tslot = inv[tgt]                        # permuted targets
    deg_slot = deg[perm].astype(np.float64)
    recip_full = (1.0 / (deg_slot + 1e-8)).astype(np.float32)
    gdeg_full = (deg_slot * recip_full).astype(np.float32)

    blk = tslot >> 7                        # permuted block id (0..127)
    order = np.lexsort((src, blk))
    src_s, tslot_s, blk_s = src[order], tslot[order], blk[order]
    counts = np.bincount(blk_s, minlength=N // 128)
    starts = np.concatenate(([0], np.cumsum(counts)))
    tpb = int(np.ceil(counts.max() / 128))
    pad = tpb * 128
    G = NB * tpb

    W1 = np.asarray(W1, np.float32)
    f8 = mybir.dt.np(f8e4)
    w1d_np = (W1[:D] - W1[D:]).astype(ml_dtypes.bfloat16)
    W1b = W1[D:]
    if qmode == "dr":
        # [128, 2, H]: [p, t, :] = FSCALE * W1b[t*128+p, :]
        w1bd_np = np.ascontiguousarray(
            (W1b * FSCALE).reshape(2, 128, H).transpose(1, 0, 2)).astype(f8)
    else:
        w1bd_np = (W1b * FSCALE).astype(f8)
    b1b_np = np.tile(np.asarray(b1, np.float32)[None, :] * FSCALE, (128, 1))
    b2b_np = np.tile(np.asarray(b2, np.float32)[None, :], (128, 1))
    # [128, 4, DO]: [p, c, :] = W2[c*128+p, :]
    w2b_np = np.ascontiguousarray(
        np.asarray(W2, np.float32).reshape(4, 128, DO).transpose(1, 0, 2)
    ).astype(ml_dtypes.bfloat16)

    x_np = np.asarray(x, np.float32)
    xt_np = np.ascontiguousarray(x_np.T.astype(ml_dtypes.bfloat16))
    # transposed x for the pre-gather (quantized once, [D, N])
    xT_q = np.ascontiguousarray(x_np.T.astype(f8))

    in_maps = []
    for c in range(NCORES):
        sflat = np.zeros((NB, pad), np.int64)
        tl = np.full((NB, pad), -1.0, np.float32)
        for b in range(NB):
            g = c * NB + b               # global (permuted) block
            s, e = starts[g], starts[g + 1]
            n = e - s
            sflat[b, :n] = src_s[s:e]
            tl[b, :n] = (tslot_s[s:e] & 127).astype(np.float32)
        # pre-gathered transposed x[src] + scatter one-hot: [128, G, 3, 128]
        # slots 0,1: [p, g, t, j] = x[src[g*128+j], t*128+p]
        # slot 2:    [e, g, 2, r] = 1 if tloc of edge g*128+e == r
        A = xT_q[:, sflat.reshape(-1)]                  # [256, G*128]
        xgt_np = np.zeros((128, G, 3, 128), f8)
        xgt_np[:, :, 0:2, :] = A.reshape(2, 128, G, 128).transpose(1, 2, 0, 3)
        tlf = tl.reshape(-1)
        valid = tlf >= 0
        cols = np.arange(G * 128)
        ev, gv = (cols[valid] & 127), (cols[valid] >> 7)
        ones = np.ones(len(ev), f8)
        xgt_np[ev, gv, 2, tlf[valid].astype(np.int64)] = ones
        # M^T tiles: [128, G*128] bf16; column g*128+e one-hot at row tloc
        mtd_np = np.zeros((128, G * 128), ml_dtypes.bfloat16)
        mtd_np[tlf[valid].astype(np.int64), cols[valid]] = 1
        recs_np = np.ascontiguousarray(
            (recip_full[c * NPC:(c + 1) * NPC] / SSCALE).reshape(NB, 128).T)
        gdeg_np = np.ascontiguousarray(
            gdeg_full[c * NPC:(c + 1) * NPC].reshape(NB, 128).T)
        in_maps.append({
            "xt_own": np.ascontiguousarray(
                xt_np[:, perm[c * NPC:(c + 1) * NPC]]),
            "w1d": w1d_np, "w1bd": w1bd_np, "b1b": b1b_np,
            "w2b": w2b_np, "b2b": b2b_np,
            "xgt": xgt_np, "mtd": mtd_np,
            "recs": recs_np, "gdeg": gdeg_np,
        })
    return tpb, in_maps, perm


def kernel(x, W1, b1, W2, b2, nn_index, k=None, _trace=False, _tmpdir=None):
    tpb, in_maps, perm = _prepare(x, W1, b1, W2, b2, nn_index)
    b2zero = bool(np.all(np.asarray(b2) == 0))
    key = (tpb, QMODE, b2zero)
    if key not in _program_cache:
        _program_cache[key] = _build(tpb, qmode=QMODE, b2zero=b2zero)
    nc = _program_cache[key]
    res = run_bass_kernel_spmd(nc, in_maps, core_ids=list(range(NCORES)),
                               trace=_trace, tmpdir=_tmpdir)
    out_perm = np.concatenate([res.results[c]["outd"] for c in range(NCORES)],
                              axis=0)
    out = np.empty_like(out_perm)
    out[perm] = out_perm                    # slot s holds node perm[s]
    if _trace:
        return out.astype(np.float32), res
    return out.astype(np.float32)


# revision 46
# speedup vs baseline: 2.7879x; 1.2025x over previous
"""Trainium2 Bass kernel for CustomDynamicEdgeConv (gnn_message_passing).

Reference computation:
    x_i = x[tgt]; x_j = x[src]
    feat = concat([x_i, x_j - x_i], -1)            # [E, 2D]
    h    = relu(feat @ W1 + b1)                    # [E, H]
    msg  = h @ W2 + b2                             # [E, Do]
    out  = segment_sum(msg, tgt) / (deg + 1e-8)

Algebraic reformulation:
    W1 = [W1a; W1b] (row split at D)
    feat @ W1 = x_i @ (W1a - W1b) + x_j @ W1b = P[tgt] + x_j @ W1b
      with P = x @ (W1a - W1b) + b1   (node-level, [N, H])
    h_e = relu(P[tgt_e] + x[src_e] @ W1b)
    out = (S @ W2) * recip + b2 * (deg * recip),  S = segment_sum(h, tgt)

Design (v7):
  * Nodes are assigned to 128-node blocks by a degree-balancing permutation
    (host side), 16 blocks per core; each core receives exactly the edges
    whose (permuted) target lies in its range — no cross-core reduction.
  * The host pre-gathers x[src] into a dense, transposed, per-tile fp8
    layout so the device does only big sequential DMA reads.
  * Q = x_j @ W1b runs as ONE fp8 DoubleRow matmul per 128-edge tile
    (contraction 256), accumulating in PSUM.
  * P stays SBUF-resident per block; P[tgt] is added into the same PSUM
    accumulation with a one-hot (M^T, bf16, host-DMA'd) matmul.
  * relu (PSUM -> SBUF fp8, x0.5 so S is scaled by 16) is split between
    DVE and ACT; h lands in [128, 2, H] pair tiles.
  * The segment sum runs on the PE as ONE fp8 DoubleRow matmul per PAIR of
    128-edge tiles (one-hot pair stationary, host-DMA'd fp8), accumulating
    S per 128-node block in PSUM.
  * fp8 scale trick: W1b and P are scaled by 32 (fp8e4 subnormal dodge);
    h is scaled by 16 = 32*0.5; 1/16 is folded into recip for the W2 stage.
  * Tail: S (bf16) goes through the hardware transpose DMA and is projected
    with W2 in dense bf16 matmuls — deferred by one block so the transpose
    latency hides under the next block's inner loop (no PE head-of-line
    stall).
  * Host un-permutes the output.
"""
import sys

sys.path.insert(0, "/opt/trn_rl_repo")

import heapq

import numpy as np
import ml_dtypes

import concourse.bacc as bacc
import concourse.mybir as mybir
from concourse.tile import TileContext
from concourse.bass_utils import run_bass_kernel_spmd

N = 16384        # nodes
D = 256          # input feature dim
H = 512          # hidden dim
DO = 256         # output dim
E = 262144       # edges
NCORES = 8
NPC = N // NCORES          # nodes per core (2048)
NB = NPC // 128            # 128-node blocks per core (16)

QMODE = "dr"               # "dr" | "f8" per-edge Q matmul mode
FSCALE = 32.0              # fp8 scale for W1b / P (power of two)
SSCALE = 16.0              # scale carried by h / S (relu scale = SSCALE/FSCALE)

f32 = mybir.dt.float32
bf16 = mybir.dt.bfloat16
f8e4 = mybir.dt.float8e4

_program_cache: dict = {}


def _build(tpb: int, reps: int = 1, ablate: frozenset = frozenset(),
           qmode: str = QMODE, rs: int = 240, sc_mode: str = "st",
           relu_pair: bool = False, hps_bufs: int = 4, b2zero: bool = True,
           ot_act: int = 1, relu_alt: bool = True):
    """Build the SPMD Bass program. tpb = padded 128-edge tiles per node block.

    reps > 1 wraps the compute in a device-side loop (benchmarking only).
    ablate: subset of {"p1","h","relu","scatter","tail"} (timing only).
    rs: relu split point — DVE does cols [:rs], ACT does [rs:].
    sc_mode: "s" = scatter to S + transpose DMAs; "st" = transposed scatter
      (4 DR matmuls per pair, S^T direct, no transpose DMAs).
    relu_pair: one relu instruction per tile pair (vs one per tile).
    b2zero: emit the cheap tail (out = o_ps * recip) — valid when b2 == 0.
    """
    nc = bacc.Bacc("TRN2")
    G = NB * tpb                       # edge tiles per core
    T2 = 2 * tpb                       # tiles per 2-block DMA batch

    xt_own = nc.dram_tensor("xt_own", [D, NPC], bf16, kind="ExternalInput")
    w1d = nc.dram_tensor("w1d", [D, H], bf16, kind="ExternalInput")
    if qmode == "dr":
        w1bd = nc.dram_tensor("w1bd", [128, 2, H], f8e4, kind="ExternalInput")
    else:
        w1bd = nc.dram_tensor("w1bd", [D, H], f8e4, kind="ExternalInput")
    # slots 0,1: x[src] halves (partition = feature); slot 2: scatter
    # one-hot (partition = edge, free = target row)
    xgt = nc.dram_tensor("xgt", [128, G, 3, 128], f8e4, kind="ExternalInput")
    b1b = nc.dram_tensor("b1b", [128, H], f32, kind="ExternalInput")
    w2b = nc.dram_tensor("w2b", [128, 4, DO], bf16, kind="ExternalInput")
    b2b = nc.dram_tensor("b2b", [128, DO], f32, kind="ExternalInput")
    mtd = nc.dram_tensor("mtd", [128, G * 128], bf16, kind="ExternalInput")
    recs = nc.dram_tensor("recs", [128, NB], f32, kind="ExternalInput")
    gdeg = nc.dram_tensor("gdeg", [128, NB], f32, kind="ExternalInput")

    outd = nc.dram_tensor("outd", [NPC, DO], f32, kind="ExternalOutput")

    relu_t = mybir.ActivationFunctionType.Relu
    mult = mybir.AluOpType.mult
    add = mybir.AluOpType.add
    amax = mybir.AluOpType.max
    dr = mybir.MatmulPerfMode.DoubleRow
    rscale = SSCALE / FSCALE

    with TileContext(nc) as tc:
        psum_left = 8 - (2 if sc_mode == "s" else 2) - 1 - 1
        hb = min(hps_bufs, psum_left * (1 if relu_pair else 2) // 2)
        with tc.tile_pool(name="const", bufs=1) as cpool, \
             tc.tile_pool(name="xgp", bufs=2) as xgp, \
             tc.tile_pool(name="mtp", bufs=2) as mtp, \
             tc.tile_pool(name="h2p", bufs=4) as h2p, \
             tc.tile_pool(name="pxp", bufs=3) as pxp, \
             tc.tile_pool(name="ssb", bufs=2) as ssbp, \
             tc.tile_pool(name="stt", bufs=12) as sttp, \
             tc.tile_pool(name="pop", bufs=6) as pop, \
             tc.tile_pool(name="hps", bufs=hb, space="PSUM") as hpsp, \
             tc.tile_pool(name="pac", bufs=1, space="PSUM") as pacp, \
             tc.tile_pool(name="ops", bufs=1, space="PSUM") as opsp, \
             tc.tile_pool(name="sps", bufs=2, space="PSUM") as spsp:

            def load_batch(b, nblk=2):
                """Issue the input DMAs for blocks [b, b+nblk)."""
                xg = xgp.tile([128, nblk * tpb, 3, 128], f8e4, tag="xg")
                nc.sync.dma_start(xg[:], xgt[:, b * tpb:(b + nblk) * tpb])
                mt_sb = mtp.tile([128, nblk * tpb * 128], bf16, tag="mt")
                nc.sync.dma_start(
                    mt_sb[:],
                    mtd[:, b * tpb * 128:(b + nblk) * tpb * 128])
                return xg, mt_sb

            prefetched = None
            w1d_sb = []
            xt_sb = []
            for kk in range(2):
                t = cpool.tile([128, H], bf16, name=f"w1d_{kk}")
                nc.sync.dma_start(t[:], w1d[kk * 128:(kk + 1) * 128, :])
                w1d_sb.append(t)
                xt = cpool.tile([128, NPC], bf16, name=f"xt_{kk}")
                nc.sync.dma_start(xt[:], xt_own[kk * 128:(kk + 1) * 128, :])
                xt_sb.append(xt)
            if qmode == "dr":
                w1b_sb = cpool.tile([128, 2, H], f8e4, name="w1b")
                nc.sync.dma_start(w1b_sb[:], w1bd[:])
            else:
                w1b_sb = []
                for kk in range(2):
                    t = cpool.tile([128, H], f8e4, name=f"w1b_{kk}")
                    nc.sync.dma_start(t[:], w1bd[kk * 128:(kk + 1) * 128, :])
                    w1b_sb.append(t)
            b1b_sb = cpool.tile([128, H], f32)
            nc.sync.dma_start(b1b_sb[:], b1b[:])
            w2_sb = cpool.tile([128, 4, DO], bf16, name="w2")
            nc.sync.dma_start(w2_sb[:], w2b[:])
            recs_sb = cpool.tile([128, NB], f32)
            nc.sync.dma_start(recs_sb[:], recs[:])
            b2b_sb = gdeg_sb = None
            if not b2zero:
                b2b_sb = cpool.tile([128, DO], f32)
                nc.sync.dma_start(b2b_sb[:], b2b[:])
                gdeg_sb = cpool.tile([128, NB], f32)
                nc.sync.dma_start(gdeg_sb[:], gdeg[:])
            zmm_sb = cpool.tile([128, 128], bf16)
            nc.vector.memset(zmm_sb[:], 0)

            if reps > 1:
                _loop = tc.For_i(0, reps, 1)
                _loop.__enter__()

            # ---- Fused block loop: P -> per-edge Q + expand -> relu ->
            # scatter (DR pairs; "s": S + transpose DMA, "st": S^T direct)
            # -> deferred W2 -> out
            if True:

                def emit_tail(b, sTs):
                    o_ps = opsp.tile([128, DO], f32, tag="o")
                    for c in range(4):
                        nc.tensor.matmul(o_ps[:], sTs[c], w2_sb[:, c],
                                         start=(c == 0), stop=(c == 3))
                    o_sb = pop.tile([128, DO], f32, tag="osb")
                    if b2zero and b % 2 < ot_act:
                        nc.scalar.mul(o_sb[:], o_ps[:], recs_sb[:, b:b + 1])
                    elif b2zero:
                        nc.vector.tensor_scalar(
                            o_sb[:], o_ps[:],
                            recs_sb[:, b:b + 1], None, mult)
                    else:
                        t1 = pop.tile([128, DO], f32, tag="t1")
                        nc.any.tensor_scalar(t1[:], b2b_sb[:],
                                             gdeg_sb[:, b:b + 1], None, mult)
                        nc.vector.scalar_tensor_tensor(
                            o_sb[:], o_ps[:], recs_sb[:, b:b + 1],
                            t1[:], mult, add)
                    nc.sync.dma_start(outd[b * 128:(b + 1) * 128, :],
                                      o_sb[:])

                def emit_sc(st_ps, s_ps, it, pm, mm, hh, stop):
                    """Emit one pair's scatter matmul(s). hh is the h2 TILE
                    (sliced here, not an AP-of-AP).

                    In "st" mode the 4 slice chains share one PSUM bank and
                    start=True resets accumulate-state at BANK granularity,
                    so the bank is zeroed by a dummy matmul up front and
                    every chain matmul runs with start=False."""
                    if sc_mode == "s":
                        rh = hh[:] if pm else hh[:, 0]
                        nc.tensor.matmul(s_ps[:], mm, rh, perf_mode=pm,
                                         start=(it == 0), stop=stop)
                    else:
                        for c in range(4):
                            lh = (hh[:, :, c * 128:(c + 1) * 128] if pm
                                  else hh[:, 0, c * 128:(c + 1) * 128])
                            nc.tensor.matmul(st_ps[:, c], lh, mm,
                                             perf_mode=pm,
                                             start=False, stop=stop,
                                             skip_group_check=True)

                pending = None
                xg = mt_sb = None
                for b in range(NB):
                    # blocks 0 and 1 load via single-block DMAs (shorter
                    # first transfer on the startup critical path); 2-block
                    # batches from b=2 on
                    if b < 2 and reps == 1:
                        xg, mt_sb = load_batch(b, nblk=1)
                        base = 0
                    else:
                        if b % 2 == 0:
                            xg, mt_sb = load_batch(b)
                        base = (b % 2) * tpb
                    # P for this block
                    p_own_b = pxp.tile([128, H], bf16, tag="pown")
                    if "p1" not in ablate:
                        acc = pacp.tile([128, H], f32, tag="acc")
                        nc.tensor.matmul(
                            acc[:], xt_sb[0][:, b * 128:(b + 1) * 128],
                            w1d_sb[0][:], start=True, stop=False)
                        nc.tensor.matmul(
                            acc[:], xt_sb[1][:, b * 128:(b + 1) * 128],
                            w1d_sb[1][:], start=False, stop=True)
                        if b % 2 == 0:
                            nc.vector.scalar_tensor_tensor(
                                p_own_b[:], acc[:], FSCALE, b1b_sb[:],
                                mult, add)
                        else:
                            nc.scalar.activation(
                                p_own_b[:], acc[:],
                                mybir.ActivationFunctionType.Copy,
                                scale=FSCALE)

                    if sc_mode == "s":
                        s_ps = spsp.tile([128, H], f32, tag="s")
                        st_ps = None
                    else:
                        st_ps = spsp.tile([128, 4, 128], f32, tag="s")
                        s_ps = None
                        if "scatter" not in ablate:
                            # zero the bank + set has_written everywhere
                            nc.tensor.matmul(st_ps[:], zmm_sb[:],
                                             w1d_sb[0][:],
                                             start=True, stop=False,
                                             skip_group_check=True)
                    h2 = None
                    h_ps = None
                    scs = []                # deferred scatter matmuls
                    for kk in range(tpb):
                        gi = base + kk
                        par = kk % 2
                        if relu_pair:
                            if par == 0:
                                h2 = h2p.tile([128, 2, H], f8e4, tag="h2")
                                h_ps = hpsp.tile([128, 2, H], f32,
                                                 tag="hps")
                            hp = h_ps[:, par]
                        else:
                            if par == 0:
                                h2 = h2p.tile([128, 2, H], f8e4, tag="h2")
                            h_ps = hpsp.tile([128, H], f32, tag="hps")
                            hp = h_ps[:]
                        if "h" not in ablate:
                            if qmode == "dr":
                                nc.tensor.matmul(
                                    hp, xg[:, gi, 0:2], w1b_sb[:],
                                    perf_mode=dr, start=True, stop=False)
                            else:
                                nc.tensor.matmul(hp, xg[:, gi, 0],
                                                 w1b_sb[0][:],
                                                 start=True, stop=False)
                                nc.tensor.matmul(hp, xg[:, gi, 1],
                                                 w1b_sb[1][:],
                                                 start=False, stop=False)
                            nc.tensor.matmul(
                                hp,
                                mt_sb[:, gi * 128:(gi + 1) * 128],
                                p_own_b[:],
                                start=False, stop=True)
                        last = (kk == tpb - 1)
                        if relu_pair:
                            if par == 1 or last:
                                hr = h2[:] if par == 1 else h2[:, 0:1]
                                pr = h_ps[:] if par == 1 else h_ps[:, 0:1]
                                if "relu" in ablate:
                                    nc.vector.memset(hr, 0)
                                else:
                                    nc.vector.tensor_scalar(
                                        hr[:, :, 0:rs], pr[:, :, 0:rs],
                                        rscale, 0.0, mult, amax)
                                    nc.scalar.activation(
                                        hr[:, :, rs:H], pr[:, :, rs:H],
                                        relu_t, scale=rscale)
                        else:
                            hsl = h2[:, par]
                            if "relu" in ablate:
                                nc.vector.memset(hsl, 0)
                            elif relu_alt:
                                # whole-tile alternation: fewer, wider ops
                                if gi % 2 == 0:
                                    nc.vector.tensor_scalar(
                                        hsl, hp, rscale, 0.0, mult, amax)
                                else:
                                    nc.scalar.activation(
                                        hsl, hp, relu_t, scale=rscale)
                            else:
                                nc.vector.tensor_scalar(
                                    hsl[:, 0:rs], hp[:, 0:rs],
                                    rscale, 0.0, mult, amax)
                                nc.scalar.activation(
                                    hsl[:, rs:H], hp[:, rs:H], relu_t,
                                    scale=rscale)
                        if "scatter" not in ablate:
                            # collect this pair's scatter; emit the PREVIOUS
                            # pair's now (deferred by one pair) so the PE
                            # queue never waits on this pair's relu
                            grew = False
                            if par == 1:
                                scs.append((xg[:, gi - 1:gi + 1, 2], h2,
                                            dr))
                                grew = True
                            elif last:
                                scs.append((xg[:, gi, 2], h2, None))
                                grew = True
                            if grew and len(scs) >= 2:
                                i = len(scs) - 2
                                mm, hh, pm = scs[i]
                                emit_sc(st_ps, s_ps, i, pm, mm, hh, False)
                    if "scatter" not in ablate and scs:
                        i = len(scs) - 1
                        mm, hh, pm = scs[i]
                        emit_sc(st_ps, s_ps, i, pm, mm, hh, True)
                    sTs = []
                    if sc_mode == "s":
                        s_sb = ssbp.tile([128, H], bf16, tag="ssb")
                        if "scatter" in ablate:
                            nc.vector.memset(s_sb[:], 0)
                        elif b % 2 == 0:
                            nc.vector.tensor_copy(s_sb[:], s_ps[:])
                        else:
                            nc.scalar.copy(s_sb[:], s_ps[:])
                        if "tail" not in ablate:
                            for c in range(4):
                                sT = sttp.tile([128, 128], bf16, tag="st")
                                nc.sync.dma_start(
                                    sT[:], s_sb[:, c * 128:(c + 1) * 128],
                                    transpose=True)
                                sTs.append(sT[:])
                    else:
                        sT4 = sttp.tile([128, 4, 128], bf16, tag="st")
                        if "scatter" in ablate:
                            nc.vector.memset(sT4[:], 0)
                        elif b % 2 == 0:
                            nc.vector.tensor_copy(sT4[:], st_ps[:])
                        else:
                            nc.scalar.copy(sT4[:], st_ps[:])
                        sTs = [sT4[:, c] for c in range(4)]
                    if "tail" not in ablate:
                        if pending is not None:
                            emit_tail(*pending)
                        pending = (b, sTs)
                if pending is not None and "tail" not in ablate:
                    emit_tail(*pending)

            if reps > 1:
                _loop.__exit__(None, None, None)

    nc.compile()
    return nc


def _balance_nodes(deg: np.ndarray):
    """Assign nodes to 128 blocks of exactly 128 nodes, balancing total degree.

    Returns perm[N]: perm[slot] = original node id; blocks 16c..16c+15
    belong to core c.
    """
    nblocks = N // 128
    order = np.argsort(-deg, kind="stable")
    heap = [(0, 0, blk) for blk in range(nblocks)]   # (edges, nodes, blk)
    heapq.heapify(heap)
    members = [[] for _ in range(nblocks)]
    for node in order:
        w = int(deg[node])
        stash = []
        while True:
            edges, nodes, blk = heapq.heappop(heap)
            if nodes < 128:
                break
            stash.append((edges, nodes, blk))
        members[blk].append(node)
        heapq.heappush(heap, (edges + w, nodes + 1, blk))
        for it in stash:
            heapq.heappush(heap, it)
    # Repair pass: swap nodes between over-full (>cap) and under-full
    # blocks so every block has at most cap = E/nblocks edges, which keeps
    # the padded tile count at its minimum.
    cap = E // nblocks
    cnt = np.array([sum(int(deg[n]) for n in m) for m in members])
    by_deg = [dict() for _ in range(nblocks)]   # blk -> {deg: [nodes]}
    for blk in range(nblocks):
        for n in members[blk]:
            by_deg[blk].setdefault(int(deg[n]), []).append(n)
    for _ in range(10000):
        hi = int(np.argmax(cnt))
        if cnt[hi] <= cap:
            break
        lo = int(np.argmin(cnt))
        want = min(cnt[hi] - cap, cap - cnt[lo])
        done = False
        for d in range(int(want), 0, -1):
            for da, nodes_a in by_deg[hi].items():
                if not nodes_a or (da - d) not in by_deg[lo]:
                    continue
                nodes_b = by_deg[lo][da - d]
                if not nodes_b:
                    continue
                a = nodes_a.pop()
                b = nodes_b.pop()
                by_deg[lo].setdefault(da, []).append(a)
                by_deg[hi].setdefault(da - d, []).append(b)
                cnt[hi] -= d
                cnt[lo] += d
                done = True
                break
            if done:
                break
        if not done:
            break
    perm = np.empty(N, np.int64)
    pos = 0
    for blk in range(nblocks):
        nodes = [n for lst in by_deg[blk].values() for n in lst]
        assert len(nodes) == 128
        perm[pos:pos + 128] = nodes
        pos += 128
    return perm


def _prepare(x, W1, b1, W2, b2, nn_index, qmode: str = QMODE):
    src = np.asarray(nn_index[0]).astype(np.int64)
    tgt = np.asarray(nn_index[1]).astype(np.int64)
    deg = np.bincount(tgt, minlength=N).astype(np.int64)

    perm = _balance_nodes(deg)              # slot -> node
    inv = np.empty(N, np.int64)             # node -> slot
    inv[perm] = np.arange(N)

    tslot = inv[tgt]                        # permuted targets
    deg_slot = deg[perm].astype(np.float64)
    recip_full = (1.0 / (deg_slot + 1e-8)).astype(np.float32)
    gdeg_full = (deg_slot * recip_full).astype(np.float32)

    blk = tslot >> 7                        # permuted block id (0..127)
    order = np.lexsort((src, blk))
    src_s, tslot_s, blk_s = src[order], tslot[order], blk[order]
    counts = np.bincount(blk_s, minlength=N // 128)
    starts = np.concatenate(([0], np.cumsum(counts)))
    tpb = int(np.ceil(counts.max() / 128))
    pad = tpb * 128
    G = NB * tpb

    W1 = np.asarray(W1, np.float32)
    f8 = mybir.dt.np(f8e4)
    w1d_np = (W1[:D] - W1[D:]).astype(ml_dtypes.bfloat16)
    W1b = W1[D:]
    if qmode == "dr":
        # [128, 2, H]: [p, t, :] = FSCALE * W1b[t*128+p, :]
        w1bd_np = np.ascontiguousarray(
            (W1b * FSCALE).reshape(2, 128, H).transpose(1, 0, 2)).astype(f8)
    else:
        w1bd_np = (W1b * FSCALE).astype(f8)
    b1b_np = np.tile(np.asarray(b1, np.float32)[None, :] * FSCALE, (128, 1))
    b2b_np = np.tile(np.asarray(b2, np.float32)[None, :], (128, 1))
    # [128, 4, DO]: [p, c, :] = W2[c*128+p, :]
    w2b_np = np.ascontiguousarray(
        np.asarray(W2, np.float32).reshape(4, 128, DO).transpose(1, 0, 2)
    ).astype(ml_dtypes.bfloat16)

    x_np = np.asarray(x, np.float32)
    xt_np = np.ascontiguousarray(x_np.T.astype(ml_dtypes.bfloat16))
    # transposed x for the pre-gather (quantized once, [D, N])
    xT_q = np.ascontiguousarray(x_np.T.astype(f8))

    in_maps = []
    for c in range(NCORES):
        sflat = np.zeros((NB, pad), np.int64)
        tl = np.full((NB, pad), -1.0, np.float32)
        for b in range(NB):
            g = c * NB + b               # global (permuted) block
            s, e = starts[g], starts[g + 1]
            n = e - s
            sflat[b, :n] = src_s[s:e]
            tl[b, :n] = (tslot_s[s:e] & 127).astype(np.float32)
        # pre-gathered transposed x[src] + scatter one-hot: [128, G, 3, 128]
        # slots 0,1: [p, g, t, j] = x[src[g*128+j], t*128+p]
        # slot 2:    [e, g, 2, r] = 1 if tloc of edge g*128+e == r
        A = xT_q[:, sflat.reshape(-1)]                  # [256, G*128]
        xgt_np = np.zeros((128, G, 3, 128), f8)
        xgt_np[:, :, 0:2, :] = A.reshape(2, 128, G, 128).transpose(1, 2, 0, 3)
        tlf = tl.reshape(-1)
        valid = tlf >= 0
        cols = np.arange(G * 128)
        ev, gv = (cols[valid] & 127), (cols[valid] >> 7)
        ones = np.ones(len(ev), f8)
        xgt_np[ev, gv, 2, tlf[valid].astype(np.int64)] = ones
        # M^T tiles: [128, G*128] bf16; column g*128+e one-hot at row tloc
        mtd_np = np.zeros((128, G * 128), ml_dtypes.bfloat16)
        mtd_np[tlf[valid].astype(np.int64), cols[valid]] = 1
        recs_np = np.ascontiguousarray(
            (recip_full[c * NPC:(c + 1) * NPC] / SSCALE).reshape(NB, 128).T)
        gdeg_np = np.ascontiguousarray(
            gdeg_full[c * NPC:(c + 1) * NPC].reshape(NB, 128).T)
        in_maps.append({
            "xt_own": np.ascontiguousarray(
                xt_np[:, perm[c * NPC:(c + 1) * NPC]]),
            "w1d": w1d_np, "w1bd": w1bd_np, "b1b": b1b_np,
            "w2b": w2b_np, "b2b": b2b_np,
            "xgt": xgt_np, "mtd": mtd_np,
            "recs": recs_np, "gdeg": gdeg_np,
        })
    return tpb, in_maps, perm


def kernel(x, W1, b1, W2, b2, nn_index, k=None, _trace=False, _tmpdir=None):
    tpb, in_maps, perm = _prepare(x, W1, b1, W2, b2, nn_index)
    b2zero = bool(np.all(np.asarray(b2) == 0))
    key = (tpb, QMODE, b2zero)
    if key not in _program_cache:
        _program_cache[key] = _build(tpb, qmode=QMODE, b2zero=b2zero)
    nc = _program_cache[key]
    res = run_bass_kernel_spmd(nc, in_maps, core_ids=list(range(NCORES)),
                               trace=_trace, tmpdir=_tmpdir)
    out_perm = np.concatenate([res.results[c]["outd"] for c in range(NCORES)],
                              axis=0)
    out = np.empty_like(out_perm)
    out[perm] = out_perm                    # slot s holds node perm[s]
    if _trace:
        return out.astype(np.float32), res
    return out.astype(np.float32)
